# revision 1
# baseline (speedup 1.0000x reference)
"""Trainium2 Bass kernel for the MACE 3-body symmetric-contraction block.

Math (equal to the reference einsum chain, validated in factor_check.py):
  1. fc1: per-l SO3 linear on irreps_x -> x[n,m,c]  (bias on l=0)
  2. Per (node n, channel c) the 9-vector x enters a symmetric polynomial:
        out[o] = sum_p w1[n,p,c] sum_i   U1[o,i,p] x_i
               + sum_p w2[n,p,c] sum_ai  U2[o,a,i,p] x_a x_i
               + sum_p w3[n,p,c] sum_abi U3[o,a,b,i,p] x_a x_b x_i
     The x-products are symmetric, so U2/U3 are pre-symmetrized and
     compressed over sorted tuples.  Monomial vector per (n,c):
        mon = [x (9) | m2 = sym x.x (45) | m3 = sym x.x.x (165)]  (219)
     One matmul D[(o,p), f] = Ucomb^T mon  (198x219) does all the
     U-contractions, batched over f = (channel, node).
  3. D * (per-node gathered element weights), p-summed with a 0/1 matmul
     -> term[o, f].
  4. w_lin and w_fc2 fused host-side into one per-l channel mix; final
     SO3 linear + bias.

Monomial/pair orderings are DIAGONAL-major so every replication needed on
device is a partition-aligned prefix/suffix copy (hardware DMA cannot
broadcast SBUF partitions):
  pairs   q=(a, a+d)        ordered by (d, a)
  triples t=(a, a+d, a+d+e) ordered by (e, d, a)
  m2[q]  = x_a * x_{a+d}:      Xpre run x[0:9-d],  Xsuf run x[d:9]
  m3[t]  = m2[(a,a+d)] * x_i:  M2rep run m2[DOFF[d]:+9-d-e], Xrep x[d+e:9]
The e-block of Xrep equals the row-suffix Xsuf[DOFF[e]:45], so Xrep needs
only 9 DMAs.  DMA instructions carry ~0.6-1.3us fixed DGE overhead, so
they are minimized and spread across SP/ACT HWDGE rings and 8 SWDGE
queues.

Sharding: pure node-parallel, 128 nodes/core on 8 cores (SPMD, no
collectives).  Host does marshaling only (gathers/transposes/casts).
"""

import numpy as np

# ---------------------------------------------------------------- constants
NTOT, MD, CD = 1024, 9, 64
P3D, P2D, P1D = 16, 4, 2
NCORES = 8
NLOC = NTOT // NCORES              # 128 nodes per core
FT = 512                           # f-tile width for the main matmul

PAIRS = [(a, a + d) for d in range(MD) for a in range(MD - d)]          # 45
TRIPS = [(a, a + d, a + d + e) for e in range(MD) for d in range(MD - e)
         for a in range(MD - d - e)]                                    # 165
N2, N3 = len(PAIRS), len(TRIPS)
DOFF = [0]
for d in range(MD):
    DOFF.append(DOFF[-1] + (MD - d))   # diag-block offsets in PAIRS
KMON = MD + N2 + N3                # 219 monomial rows
MOUT = MD * (P3D + P2D + P1D)      # 198 output rows of the main matmul
MSPLIT = 128                       # k/out split point
M3SPLIT = MSPLIT - MD - N2         # 74: m3 rows below this live in mon_lo

# mon_lo row layout: [m3[0:74] | x (9) | m2 (45)]; mon_hi = m3[74:165].
# KPERM maps canonical Ucomb k-rows (x|m2|m3) to this layout.
KPERM = list(range(54, 128)) + list(range(0, 9)) + list(range(9, 54))
XR = 74                            # x rows start in mon_lo
M2R = 83                           # m2 rows start in mon_lo

# packed fp16 constant layout (columns); xt/ysb/termT widths scale w/ nloc
W9 = MD * NLOC
_PK_ITEMS = (("uclo", MOUT), ("uchi", MOUT), ("glo", MD), ("ghi", MD),
             ("w1t", 3 * CD), ("wct", 3 * CD))
PK_OFF = {}
_c = 0
for _nm, _w in _PK_ITEMS:
    PK_OFF[_nm] = _c
    _c += _w
PK_BASE = _c                       # xt starts here

_PROGRAM = {}                      # (nloc, repeat) -> compiled Bass program


# ---------------------------------------------------------------- host prep
def _sym_compress(U3, U2):
    tidx = {t: k for k, t in enumerate(TRIPS)}
    qidx = {q: k for k, q in enumerate(PAIRS)}
    U3c = np.zeros((MD, N3, P3D), np.float64)
    for a in range(MD):
        for b in range(MD):
            for i in range(MD):
                U3c[:, tidx[tuple(sorted((a, b, i)))], :] += U3[:, a, b, i, :]
    U2c = np.zeros((MD, N2, P2D), np.float64)
    for a in range(MD):
        for i in range(MD):
            U2c[:, qidx[tuple(sorted((a, i)))], :] += U2[:, a, i, :]
    return U3c.astype(np.float32), U2c.astype(np.float32)


def _build_ucomb_g(U3c, U2c, U1):
    # Ucomb[k, mo]; mon rows: 0..8 x | 9..53 m2 | 54..218 m3
    # out rows mo:  o*16+p (corr3) | 144+o*4+p (corr2) | 180+o*2+p (corr1)
    U = np.zeros((KMON, MOUT), np.float32)
    G = np.zeros((MOUT, MD), np.float32)
    for o in range(MD):
        U[54:54 + N3, o * P3D:(o + 1) * P3D] = U3c[o]
        U[9:9 + N2, 144 + o * P2D:144 + (o + 1) * P2D] = U2c[o]
        U[0:MD, 180 + o * P1D:180 + (o + 1) * P1D] = U1[o]
        G[o * P3D:(o + 1) * P3D, o] = 1.0
        G[144 + o * P2D:144 + (o + 1) * P2D, o] = 1.0
        G[180 + o * P1D:180 + (o + 1) * P1D, o] = 1.0
    return U, G


# ---------------------------------------------------------------- device
def _build_program(nloc, repeat=1, stage=4):
    import concourse.bacc as bacc
    import concourse.bass as bass
    from concourse import mybir
    from concourse.tile import TileContext

    f16 = mybir.dt.float16
    f32 = mybir.dt.float32
    AF = mybir.ActivationFunctionType
    F = nloc * CD
    nft = max(1, F // FT)
    ft = min(FT, F)
    w9 = MD * nloc
    lblk = [(0, nloc), (nloc, 4 * nloc), (4 * nloc, 9 * nloc)]
    pkw_in = PK_BASE + w9              # loaded region of the pack
    pkw = PK_BASE + 3 * w9             # + device-only ysb, termT

    nc = bacc.Bacc("TRN2", debug=False, enable_asserts=False,
                   num_devices=NCORES, num_swdge_queues=4)

    pk_d = nc.dram_tensor("pk", [128, pkw_in], f16, kind="ExternalInput").ap()
    b12_d = nc.dram_tensor("b12", [CD, 2], f32, kind="ExternalInput").ap()
    wg_d = nc.dram_tensor("wg", [P3D + P2D + P1D, F], f16,
                          kind="ExternalInput").ap()
    out_d = nc.dram_tensor("out", [CD, w9], f32, kind="ExternalOutput").ap()

    with TileContext(nc) as tc:
        with (
            tc.tile_pool(name="const", bufs=1) as const,
            tc.tile_pool(name="big", bufs=1) as big,
            tc.tile_pool(name="work", bufs=2) as work,
            tc.tile_pool(name="ps_a", bufs=1, space="PSUM") as ps_a,
            tc.tile_pool(name="ps_d", bufs=2, space="PSUM") as ps_d,
            tc.tile_pool(name="ps_t", bufs=2, space="PSUM") as ps_t,
        ):
          # DMA spreader: SP + ACT HWDGE rings, then 8 SWDGE queues
          _dmaq = [0]

          def dma(out, in_, small=False):
              if small:
                  nc.gpsimd.dma_start(out=out, in_=in_)
                  return
              i = _dmaq[0] % 2
              _dmaq[0] += 1
              if i == 0:
                  nc.sync.dma_start(out=out, in_=in_)
              else:
                  nc.scalar.dma_start(out=out, in_=in_)

          def _emit():
            ox = PK_BASE
            oy = PK_BASE + w9
            ot = PK_BASE + 2 * w9
            pk = const.tile([128, pkw], f16)
            sb_uclo = pk[0:128, PK_OFF["uclo"]:PK_OFF["uclo"] + MOUT]
            sb_uchi = pk[0:KMON - 128, PK_OFF["uchi"]:PK_OFF["uchi"] + MOUT]
            sb_glo = pk[0:128, PK_OFF["glo"]:PK_OFF["glo"] + MD]
            sb_ghi = pk[0:MOUT - 128, PK_OFF["ghi"]:PK_OFF["ghi"] + MD]
            sb_w1t = pk[0:CD, PK_OFF["w1t"]:PK_OFF["w1t"] + 3 * CD]
            sb_wct = pk[0:CD, PK_OFF["wct"]:PK_OFF["wct"] + 3 * CD]
            sb_xt = pk[0:CD, ox:ox + w9]
            ysb = pk[0:CD, oy:oy + w9]
            termT = pk[0:CD, ot:ot + w9]
            pk32 = const.tile([CD, 2 + w9], f32)
            sb_b1 = pk32[:, 0:1]
            sb_b2 = pk32[:, 1:2]
            outSB = pk32[:, 2:2 + w9]
            nc.sync.dma_start(out=pk[:, 0:pkw_in], in_=pk_d)
            nc.scalar.dma_start(out=pk32[:, 0:2], in_=b12_d)

            # ---------------- S2: fc1 per l (cols m-major within l block)
            for l, (c0, c1) in enumerate(lblk):
                w_l = sb_w1t[:, l * CD:(l + 1) * CD]
                for s0 in range(c0, c1, FT):
                    s1 = min(s0 + FT, c1)
                    py = ps_a.tile([CD, ft], mybir.dt.float32, name="py",
                                   tag="py", bufs=1)
                    nc.tensor.matmul(py[:, :s1 - s0], lhsT=w_l,
                                     rhs=sb_xt[:, s0:s1], start=True, stop=True)
                    if l == 0:
                        nc.scalar.activation(ysb[:, s0:s1], py[:, :s1 - s0],
                                             AF.Identity, bias=sb_b1)
                    else:
                        nc.scalar.activation(ysb[:, s0:s1], py[:, :s1 - s0],
                                             AF.Copy)

            # ---------------- monomial tiles: mon_lo = [m3a | x | m2]
            mon_lo = big.tile([128, F], f16)
            mon_hi = big.tile([KMON - 128, F], f16)

            # S3: x rows — each m is a contiguous [64, nloc] slab of ysb
            for l, (c0, c1) in enumerate(lblk):
                m0 = l * l
                for mm in range(2 * l + 1):
                    src = ysb[:, c0 + mm * nloc:c0 + (mm + 1) * nloc]
                    dma(mon_lo[XR + m0 + mm:XR + m0 + mm + 1, :], src,
                        small=True)

            if stage <= 1:
                nc.sync.dma_start(out=out_d, in_=outSB)
                return
            # S4a: m2 = Xpre * Xsuf (diag-major pairs) -> m2t -> mon_lo[83:]
            xpre = big.tile([N2, F], f16, tag="shA")
            xsuf = big.tile([N2, F], f16, tag="shB")
            for d in range(MD):
                q0, cnt = DOFF[d], MD - d
                dma(xpre[q0:q0 + cnt, :], mon_lo[XR:XR + cnt, :])
                dma(xsuf[q0:q0 + cnt, :], mon_lo[XR + d:XR + MD, :])
            m2t = big.tile([N2, F], f16, tag="shT")
            nc.vector.tensor_mul(m2t[:], xpre[:], xsuf[:])
            dma(mon_lo[M2R:M2R + N2, :], m2t[:])

            # S4b: m3 runs (e, d): m2[DOFF[d]:+len] * x[d+e:9], len = 9-d-e
            m2rep_a = big.tile([M3SPLIT, F], f16, tag="shA")
            m2rep_b = big.tile([N3 - M3SPLIT, F], f16, tag="shB")
            xrep_a = big.tile([M3SPLIT, F], f16, tag="shC")
            xrep_b = big.tile([N3 - M3SPLIT, F], f16, tag="shD")

            def run_copy(dst_a, dst_b, t0, src, s0, ln):
                if t0 < M3SPLIT:
                    n_lo = min(ln, M3SPLIT - t0)
                    dma(dst_a[t0:t0 + n_lo, :], src[s0:s0 + n_lo, :])
                    if n_lo < ln:
                        dma(dst_b[0:ln - n_lo, :], src[s0 + n_lo:s0 + ln, :])
                else:
                    dma(dst_b[t0 - M3SPLIT:t0 - M3SPLIT + ln, :],
                        src[s0:s0 + ln, :])

            # m2rep: e=0 block is all of m2; e>0 blocks are per-(e,d) prefixes
            t0 = N2
            run_copy(m2rep_a, m2rep_b, 0, m2t, 0, N2)
            for e in range(1, MD):
                for d in range(MD - e):
                    ln = MD - d - e
                    run_copy(m2rep_a, m2rep_b, t0, m2t, DOFF[d], ln)
                    t0 += ln
            # xrep: e block == xsuf[DOFF[e]:45]  (one suffix copy per e)
            t0 = 0
            for e in range(MD):
                ln = N2 - DOFF[e]
                run_copy(xrep_a, xrep_b, t0, xsuf, DOFF[e], ln)
                t0 += ln
            nc.vector.tensor_mul(mon_lo[0:M3SPLIT, :], m2rep_a[:], xrep_a[:])
            nc.vector.tensor_mul(mon_hi[:], m2rep_b[:], xrep_b[:])

            if stage <= 2:
                nc.sync.dma_start(out=out_d, in_=outSB)
                return
            # ---------------- S5: wrep — 4 DMAs with step-0 DRAM source
            wrep_lo = big.tile([128, F], f16, tag="shC")
            wrep_hi = big.tile([MOUT - 128, F], f16, tag="shD")
            src8 = bass.AP(tensor=wg_d.tensor, offset=0,
                           ap=[[0, 8], [F, 16], [1, F]])
            nc.sync.dma_start(out=wrep_lo[:], in_=src8)
            nc.scalar.dma_start(out=wrep_hi[0:16, :], in_=wg_d[0:16, :])
            src2 = bass.AP(tensor=wg_d.tensor, offset=16 * F,
                           ap=[[0, MD], [F, 4], [1, F]])
            nc.sync.dma_start(out=wrep_hi[16:52, :], in_=src2)
            src1 = bass.AP(tensor=wg_d.tensor, offset=20 * F,
                           ap=[[0, MD], [F, 2], [1, F]])
            nc.scalar.dma_start(out=wrep_hi[52:70, :], in_=src1)

            if stage <= 3:
                nc.sync.dma_start(out=out_d, in_=outSB)
                return
            # ---------------- S6/S7: main matmul + weight apply + group sum
            termSB = big.tile([MD, F], f16, tag="shT")
            for j in range(nft):
                js = slice(j * ft, (j + 1) * ft)
                dlo = ps_d.tile([128, ft], mybir.dt.float32, name="dlo", tag="dlo")
                dhi = ps_d.tile([MOUT - 128, ft], mybir.dt.float32, name="dhi",
                                tag="dhi")
                nc.tensor.matmul(dlo[:], lhsT=sb_uclo[:, 0:128],
                                 rhs=mon_lo[:, js], start=True, stop=False)
                nc.tensor.matmul(dlo[:], lhsT=sb_uchi[:, 0:128],
                                 rhs=mon_hi[:, js], start=False, stop=True)
                nc.tensor.matmul(dhi[:], lhsT=sb_uclo[:, 128:MOUT],
                                 rhs=mon_lo[:, js], start=True, stop=False)
                nc.tensor.matmul(dhi[:], lhsT=sb_uchi[:, 128:MOUT],
                                 rhs=mon_hi[:, js], start=False, stop=True)
                dw_lo = work.tile([128, ft], f16, name="dw_lo", tag="dw_lo")
                dw_hi = work.tile([MOUT - 128, ft], f16, name="dw_hi", tag="dw_hi")
                nc.vector.tensor_mul(dw_lo[:], dlo[:], wrep_lo[:, js])
                nc.vector.tensor_mul(dw_hi[:], dhi[:], wrep_hi[:, js])
                pt = ps_t.tile([MD, ft], mybir.dt.float32, name="pt", tag="pt")
                nc.tensor.matmul(pt[:], lhsT=sb_glo, rhs=dw_lo[:],
                                 start=True, stop=False)
                nc.tensor.matmul(pt[:], lhsT=sb_ghi, rhs=dw_hi[:],
                                 start=False, stop=True)
                nc.scalar.activation(termSB[:, js], pt[:], AF.Copy)

            # ---------------- S8: termT[c, (o,n)] one DMA per o
            for o in range(MD):
                dma(termT[:, o * nloc:(o + 1) * nloc], termSB[o:o + 1, :],
                    small=True)

            # final fused (w_fc2 @ w_lin) SO3 linear; cols (o, n) o-major
            for l, (c0, c1) in enumerate(lblk):
                w_l = sb_wct[:, l * CD:(l + 1) * CD]
                for s0 in range(c0, c1, FT):
                    s1 = min(s0 + FT, c1)
                    pf = ps_a.tile([CD, ft], mybir.dt.float32, name="pf",
                                   tag="pf", bufs=1)
                    nc.tensor.matmul(pf[:, :s1 - s0], lhsT=w_l,
                                     rhs=termT[:, s0:s1], start=True, stop=True)
                    if l == 0:
                        nc.scalar.activation(outSB[:, s0:s1], pf[:, :s1 - s0],
                                             AF.Identity, bias=sb_b2)
                    else:
                        nc.scalar.activation(outSB[:, s0:s1], pf[:, :s1 - s0],
                                             AF.Copy)
            nc.sync.dma_start(out=out_d, in_=outSB)

          if repeat > 1:
              with tc.For_i(0, repeat, 1):
                  _emit()
          else:
              _emit()

    return nc


def _get_program(nloc, repeat=1, stage=4):
    key = (nloc, repeat, stage)
    if key not in _PROGRAM:
        nc = _build_program(nloc, repeat, stage)
        nc.compile()
        _PROGRAM[key] = nc
    return _PROGRAM[key]


def make_in_maps(irreps_x, atomic_numbers, w_fc1, b_fc1, U3, W3, U2, W2, U1, W1,
                 w_lin, w_fc2, b_fc2, nloc=NLOC, ncores=NCORES):
    irreps_x = np.asarray(irreps_x, np.float32)
    a_n = np.asarray(atomic_numbers).astype(np.int64)
    U3c, U2c = _sym_compress(np.asarray(U3, np.float64),
                             np.asarray(U2, np.float64))
    Ucomb, G = _build_ucomb_g(U3c, U2c, np.asarray(U1, np.float32))
    w_comb = np.einsum('lde,lec->ldc', np.asarray(w_fc2, np.float32),
                       np.asarray(w_lin, np.float32))
    w1t = np.concatenate([np.asarray(w_fc1, np.float32)[l].T for l in range(3)],
                         axis=1)
    wct = np.concatenate([w_comb[l].T for l in range(3)], axis=1)
    w3g = np.asarray(W3, np.float32)[a_n]
    w2g = np.asarray(W2, np.float32)[a_n]
    w1g = np.asarray(W1, np.float32)[a_n]
    F = nloc * CD
    w9 = MD * nloc
    pkw_in = PK_BASE + w9

    def put(buf, nm, arr):
        o = PK_OFF[nm]
        arr = np.asarray(arr, np.float32).astype(np.float16)
        buf[:arr.shape[0], o:o + arr.shape[1]] = arr

    uc_p = Ucomb[KPERM]
    b12 = np.stack([np.asarray(b_fc1, np.float32),
                    np.asarray(b_fc2, np.float32)], axis=1).astype(np.float32)
    in_maps = []
    for core in range(ncores):
        s = slice(core * nloc, (core + 1) * nloc)
        parts = []
        for l in range(3):
            seg = irreps_x[s, l * l:(l + 1) * (l + 1), :]   # [nloc, w, 64]
            parts.append(seg.transpose(2, 1, 0).reshape(CD, -1))
        xt = np.concatenate(parts, axis=1)                  # [64, 9*nloc]
        pk = np.zeros((128, pkw_in), np.float16)
        put(pk, "uclo", uc_p[0:128])
        put(pk, "uchi", Ucomb[128:KMON])
        put(pk, "glo", G[0:128])
        put(pk, "ghi", G[128:MOUT])
        put(pk, "w1t", w1t)
        put(pk, "wct", wct)
        pk[:CD, PK_BASE:PK_BASE + w9] = xt.astype(np.float16)
        wg = np.concatenate([
            w3g[s].transpose(1, 2, 0).reshape(P3D, F),
            w2g[s].transpose(1, 2, 0).reshape(P2D, F),
            w1g[s].transpose(1, 2, 0).reshape(P1D, F),
        ], axis=0)                                          # [22, F] f=c*nloc+n
        in_maps.append({
            "pk": pk,
            "b12": b12,
            "wg": wg.astype(np.float16),
        })
    return in_maps


def unpack_out(o, nloc=NLOC):
    # o: [64, 9*nloc] cols (o, n) o-major -> [nloc, 9, 64]
    return np.ascontiguousarray(
        o.reshape(CD, MD, nloc).transpose(2, 1, 0)).astype(np.float32)


# ---------------------------------------------------------------- entry
def kernel(**inputs):
    from concourse import bass_utils
    in_maps = make_in_maps(**inputs)
    nc = _get_program(NLOC)
    res = bass_utils.run_bass_kernel_spmd(nc, in_maps,
                                          core_ids=list(range(NCORES)))
    outs = [unpack_out(res.results[c]["out"]) for c in range(NCORES)]
    return np.concatenate(outs, axis=0).astype(np.float32)



# revision 56
# speedup vs baseline: 2.9058x; 2.9058x over previous
"""Trainium2 Bass kernel for the MACE 3-body symmetric-contraction block.

Math (equal to the reference einsum chain, validated in factor_check.py):
  1. fc1: per-l SO3 linear on irreps_x -> x[n,m,c]  (bias on l=0)
  2. Per (node n, channel c) the 9-vector x enters a symmetric polynomial:
        out[o] = sum_p w1[n,p,c] sum_i   U1[o,i,p] x_i
               + sum_p w2[n,p,c] sum_ai  U2[o,a,i,p] x_a x_i
               + sum_p w3[n,p,c] sum_abi U3[o,a,b,i,p] x_a x_b x_i
     The x-products are symmetric, so U2/U3 are pre-symmetrized and
     compressed over sorted tuples.  Monomial vector per (n,c):
        mon = [x (9) | m2 = sym x.x (45) | m3 = sym x.x.x (165)]  (219)
     One matmul D[(o,p), f] = Ucomb^T mon  (198x219) does all the
     U-contractions, batched over f = (channel, node).
  3. D * (per-node gathered element weights), p-summed with a 0/1 matmul
     -> term[o, f].
  4. w_lin and w_fc2 fused host-side into one per-l channel mix; final
     SO3 linear + bias.

Monomial/pair orderings are DIAGONAL-major so every replication needed on
device is a partition-aligned prefix/suffix copy (hardware DMA cannot
broadcast SBUF partitions):
  pairs   q=(a, a+d)        ordered by (d, a)
  triples t=(a, a+d, a+d+e) ordered by (e, d, a)
  m2[q]  = x_a * x_{a+d}:      Xpre run x[0:9-d],  Xsuf run x[d:9]
  m3[t]  = m2[(a,a+d)] * x_i:  m2rep run m2[DOFF[d]:+9-d-e], Xrep x[d+e:9]
The e-block of Xrep equals the row-suffix Xsuf[DOFF[e]:45], so Xrep needs
only 9 DMAs.  The 37 m2rep prefix runs are NOT DMA'd: a 0/1 gather matrix
(srep) replays them as a PE selection matmul per 512-column chunk, with
one ACT op moving both PSUM halves to SBUF f16 and DVE forming the m3
chunk — all fused into a 6-stage software pipeline with the main
U-contraction / weight / group-sum chain (no_sync_barrier pins the
schedule; stages are skewed so each block only consumes previous-block
results).  Remaining DMAs are split across the HWDGE rings and SWDGE.

Sharding: pure node-parallel, 128 nodes/core on 8 cores (SPMD, no
collectives).  Host does marshaling only (gathers/transposes/casts).
"""

import numpy as np

# ---------------------------------------------------------------- constants
NTOT, MD, CD = 1024, 9, 64
P3D, P2D, P1D = 16, 4, 2
NCORES = 8
NLOC = NTOT // NCORES              # 128 nodes per core
FT = 512                           # f-tile width for the main matmul

PAIRS = [(a, a + d) for d in range(MD) for a in range(MD - d)]          # 45
TRIPS = [(a, a + d, a + d + e) for e in range(MD) for d in range(MD - e)
         for a in range(MD - d - e)]                                    # 165
N2, N3 = len(PAIRS), len(TRIPS)
DOFF = [0]
for d in range(MD):
    DOFF.append(DOFF[-1] + (MD - d))   # diag-block offsets in PAIRS
KMON = MD + N2 + N3                # 219 monomial rows
MOUT = MD * (P3D + P2D + P1D)      # 198 output rows of the main matmul
MSPLIT = 128                       # k/out split point
M3SPLIT = 64                       # m3 rows below this live in mon_lo

# mon_lo row layout: [m3a (64) | m2 (45) | x (9) | zeros (10)];
# mon_hi = m3[64:165] (101 rows). Compute-engine APs need 32-aligned base
# partitions and matmul needs lhsT.base == rhs.base, so m2 sits at 64 (as
# do srep in pk and xpre/xsuf in their tiles) and m3a at 0. The 10 zero
# rows pair with zero Ucomb rows (filled once from a zero DRAM row).
KPERM = list(range(54, 118)) + list(range(9, 54)) + list(range(0, 9))
M3A0 = 0                           # m3a rows start in mon_lo
M2R = 64                           # m2 rows start in mon_lo
XR = 109                           # x rows start in mon_lo
ZR = 118                           # zero rows start in mon_lo

# packed fp16 constant layout (columns); xt/ysb/termT widths scale w/ nloc
W9 = MD * NLOC
_PK_ITEMS = (("w1t", 3 * CD), ("uclo", MOUT), ("uchi", MOUT), ("glo", MD),
             ("ghi", MD), ("wct", 3 * CD),
             ("srepa", N3 - M3SPLIT), ("srepb", N3 - M3SPLIT))
PK_OFF = {}
_c = 0
for _nm, _w in _PK_ITEMS:
    PK_OFF[_nm] = _c
    _c += _w
PK_BASE = _c                       # xt starts here

_PROGRAM = {}                      # (nloc, repeat) -> compiled Bass program


# ---------------------------------------------------------------- host prep
def _sym_compress(U3, U2):
    tidx = {t: k for k, t in enumerate(TRIPS)}
    qidx = {q: k for k, q in enumerate(PAIRS)}
    U3c = np.zeros((MD, N3, P3D), np.float64)
    for a in range(MD):
        for b in range(MD):
            for i in range(MD):
                U3c[:, tidx[tuple(sorted((a, b, i)))], :] += U3[:, a, b, i, :]
    U2c = np.zeros((MD, N2, P2D), np.float64)
    for a in range(MD):
        for i in range(MD):
            U2c[:, qidx[tuple(sorted((a, i)))], :] += U2[:, a, i, :]
    return U3c.astype(np.float32), U2c.astype(np.float32)


def _build_ucomb_g(U3c, U2c, U1):
    # Ucomb[k, mo]; mon rows: 0..8 x | 9..53 m2 | 54..218 m3
    # out rows mo:  o*16+p (corr3) | 144+o*4+p (corr2) | 180+o*2+p (corr1)
    U = np.zeros((KMON, MOUT), np.float32)
    G = np.zeros((MOUT, MD), np.float32)
    for o in range(MD):
        U[54:54 + N3, o * P3D:(o + 1) * P3D] = U3c[o]
        U[9:9 + N2, 144 + o * P2D:144 + (o + 1) * P2D] = U2c[o]
        U[0:MD, 180 + o * P1D:180 + (o + 1) * P1D] = U1[o]
        G[o * P3D:(o + 1) * P3D, o] = 1.0
        G[144 + o * P2D:144 + (o + 1) * P2D, o] = 1.0
        G[180 + o * P1D:180 + (o + 1) * P1D, o] = 1.0
    return U, G


# ---------------------------------------------------------------- device
def _build_program(nloc, repeat=1, stage=4):
    import concourse.bacc as bacc
    import concourse.bass as bass
    from concourse import mybir
    from concourse.tile import TileContext

    f16 = mybir.dt.float16
    f32 = mybir.dt.float32
    AF = mybir.ActivationFunctionType
    F = nloc * CD
    nft = max(1, F // FT)
    ft = min(FT, F)
    w9 = MD * nloc
    lblk = [(0, nloc), (nloc, 4 * nloc), (4 * nloc, 9 * nloc)]
    pkw_in = PK_BASE + w9              # loaded region of the pack

    nc = bacc.Bacc("TRN2", debug=False, enable_asserts=False,
                   num_devices=NCORES, num_swdge_queues=4)

    pk_d = nc.dram_tensor("pk", [128, pkw_in], f16, kind="ExternalInput").ap()
    b12_d = nc.dram_tensor("b12", [CD, 2], f32, kind="ExternalInput").ap()
    wg_d = nc.dram_tensor("wg", [P3D + P2D + P1D, F], f16,
                          kind="ExternalInput").ap()
    z_d = nc.dram_tensor("z", [1, F], f16, kind="ExternalInput").ap()
    out_d = nc.dram_tensor("out", [CD, w9], f32, kind="ExternalOutput").ap()

    with TileContext(nc) as tc:
        with (
            tc.tile_pool(name="const", bufs=2) as const,
            tc.tile_pool(name="big", bufs=1) as big,
            tc.tile_pool(name="work", bufs=2) as work,
            tc.tile_pool(name="ps_s", bufs=1, space="PSUM") as ps_s,
            tc.tile_pool(name="ps_d", bufs=2, space="PSUM") as ps_d,
            tc.tile_pool(name="ps_t", bufs=2, space="PSUM") as ps_t,
        ):
          # explicit queue spreader: 0 = SP ring, 1 = ACT ring, 2 = SWDGE
          def dma(q, out, in_):
              if q == 0:
                  nc.sync.dma_start(out=out, in_=in_)
              elif q == 1:
                  nc.scalar.dma_start(out=out, in_=in_)
              else:
                  nc.gpsimd.dma_start(out=out, in_=in_)

          def _emit():
            pk = const.tile([128, pkw_in], f16)
            sb_uclo = pk[0:128, PK_OFF["uclo"]:PK_OFF["uclo"] + MOUT]
            sb_uchi = pk[0:N3 - M3SPLIT, PK_OFF["uchi"]:PK_OFF["uchi"] + MOUT]
            sb_glo = pk[0:128, PK_OFF["glo"]:PK_OFF["glo"] + MD]
            sb_ghi = pk[0:MOUT - 128, PK_OFF["ghi"]:PK_OFF["ghi"] + MD]
            sb_w1t = pk[0:CD, PK_OFF["w1t"]:PK_OFF["w1t"] + 3 * CD]
            sb_wct = pk[0:CD, PK_OFF["wct"]:PK_OFF["wct"] + 3 * CD]
            nhr = N3 - M3SPLIT
            sb_srepa = pk[M2R:M2R + N2,
                          PK_OFF["srepa"]:PK_OFF["srepa"] + nhr]
            sb_srepb = pk[M2R:M2R + N2,
                          PK_OFF["srepb"]:PK_OFF["srepb"] + nhr]
            sb_xt = pk[0:CD, PK_BASE:PK_BASE + w9]
            ysb = const.tile([CD, w9], f16)
            termT = const.tile([CD, w9], f16)
            pk32 = const.tile([CD, 2 + w9], f32)
            sb_b1 = pk32[:, 0:1]
            sb_b2 = pk32[:, 1:2]
            outSB = pk32[:, 2:2 + w9]

            mon_lo = big.tile([128, F], f16)
            mon_hi = big.tile([N3 - M3SPLIT, F], f16)
            xpre = big.tile([M2R + N2, F], f16)
            xsuf = big.tile([M2R + N2, F], f16)
            # m2rep halves side by side in one tile: cols [0:F) hold the
            # 64-row a-half, [F:2F) the 101-row b-half, so one ACT op can
            # copy both sel results per chunk.
            m2rep = big.tile([N3 - M3SPLIT, 2 * F], f16)
            xrep_a = big.tile([M3SPLIT, F], f16)
            xrep_b = big.tile([N3 - M3SPLIT, F], f16)
            wrep_lo = big.tile([128, F], f16)
            wrep_hi = big.tile([MOUT - 128, F], f16)
            termSB = big.tile([MD, F], f16)

            # t=0 loads. wrep is NOT loaded here: its 3.2MB would hog the
            # DMA-data slot and push back S3/xsuf completions; it is only
            # needed by the first dw mul, so it loads during the copy phase.
            # pk in three pieces: w1t + xt first so fc1 starts ~1us earlier
            nc.sync.dma_start(out=pk[:, 0:3 * CD], in_=pk_d[:, 0:3 * CD])
            nc.sync.dma_start(out=pk[:, PK_BASE:pkw_in],
                              in_=pk_d[:, PK_BASE:pkw_in])
            nc.sync.dma_start(out=pk[:, 3 * CD:PK_BASE],
                              in_=pk_d[:, 3 * CD:PK_BASE])
            nc.scalar.dma_start(out=pk32[:, 0:2], in_=b12_d)

            # ---------------- S2: fc1 per l (cols m-major within l block)
            for l, (c0, c1) in enumerate(lblk):
                w_l = sb_w1t[:, l * CD:(l + 1) * CD]
                for s0 in range(c0, c1, FT):
                    s1 = min(s0 + FT, c1)
                    py = ps_t.tile([CD, ft], mybir.dt.float32, name="py",
                                   tag="pt")
                    nc.tensor.matmul(py[:, :s1 - s0], lhsT=w_l,
                                     rhs=sb_xt[:, s0:s1], start=True, stop=True)
                    if l == 0:
                        nc.scalar.activation(ysb[:, s0:s1], py[:, :s1 - s0],
                                             AF.Identity, bias=sb_b1)
                    else:
                        nc.scalar.activation(ysb[:, s0:s1], py[:, :s1 - s0],
                                             AF.Copy)

            # S3: x rows — each m is a contiguous [64, nloc] slab of ysb.
            # SP ring + SWDGE only: the ACT sequencer must stay free to run
            # the ysb activations these DMAs wait on (head-of-line blocking).
            for m in range(MD):
                src = ysb[:, m * nloc:(m + 1) * nloc]
                dma(2 if m % 2 else 0, mon_lo[XR + m:XR + m + 1, :], src)

            if stage <= 1:
                nc.sync.dma_start(out=out_d, in_=outSB)
                return
            # zero rows of mon_lo (pair with zero Ucomb rows)
            zsrc = bass.AP(tensor=z_d.tensor, offset=0,
                           ap=[[0, 128 - ZR], [1, F]])
            nc.scalar.dma_start(out=mon_lo[ZR:128, :], in_=zsrc)

            # S4a: xsuf/xpre (based at partition 64, like m2); the HWDGE
            # lane is ~0.63us/DMA vs SWDGE ~1us, so give it the larger share
            _q = [0, 1, 2, 0, 1]               # 4:1 HWDGE:SWDGE
            for d in range(MD):
                q0, cnt = M2R + DOFF[d], MD - d
                dma(_q[(2 * d) % 5], xsuf[q0:q0 + cnt, :],
                    mon_lo[XR + d:XR + MD, :])
                dma(_q[(2 * d + 1) % 5], xpre[q0:q0 + cnt, :],
                    mon_lo[XR:XR + cnt, :])

            # xrep: e block == xsuf[DOFF[e]:45] — SP ring + SWDGE
            xrep_runs = []
            t0 = 0
            for e in range(MD):
                ln = N2 - DOFF[e]
                xrep_runs.append((t0, DOFF[e], ln))
                t0 += ln
            qi = 0
            for t0, s0, ln in xrep_runs:
                segs = []
                if t0 < M3SPLIT:
                    n_lo = min(ln, M3SPLIT - t0)
                    segs.append((xrep_a, t0, s0, n_lo))
                    if n_lo < ln:
                        segs.append((xrep_b, 0, s0 + n_lo, ln - n_lo))
                else:
                    segs.append((xrep_b, t0 - M3SPLIT, s0, ln))
                for dst, dt0, ds0, dln in segs:
                    dma([0, 2][qi % 2], dst[dt0:dt0 + dln, :],
                        xsuf[M2R + ds0:M2R + ds0 + dln, :])
                    qi += 1

            # wrep — 4 DMAs with step-0 DRAM source; first needed by dw(j=0)
            src8 = bass.AP(tensor=wg_d.tensor, offset=0,
                           ap=[[0, 8], [F, 16], [1, F]])
            nc.sync.dma_start(out=wrep_lo[:], in_=src8)
            nc.scalar.dma_start(out=wrep_hi[0:16, :], in_=wg_d[0:16, :])
            src2 = bass.AP(tensor=wg_d.tensor, offset=16 * F,
                           ap=[[0, MD], [F, 4], [1, F]])
            nc.sync.dma_start(out=wrep_hi[16:52, :], in_=src2)
            src1 = bass.AP(tensor=wg_d.tensor, offset=20 * F,
                           ap=[[0, MD], [F, 2], [1, F]])
            nc.scalar.dma_start(out=wrep_hi[52:70, :], in_=src1)

            # m2 written straight into its mon_lo slot
            # column quarters so the first sel chunks start earlier
            m2 = mon_lo[M2R:M2R + N2, :]
            fq = F // 4
            for qq in range(4):
                qs = slice(qq * fq, (qq + 1) * fq)
                nc.vector.tensor_mul(m2[:, qs], xpre[M2R:M2R + N2, qs],
                                     xsuf[M2R:M2R + N2, qs])

            if stage <= 2:
                nc.sync.dma_start(out=out_d, in_=outSB)
                return
            # ---------------- fused, software-pipelined main loop over
            # 512-col chunks. Stage A(j): m2rep via PE selection matmul
            # (srep is a 0/1 gather matrix replacing 37 SBUF->SBUF copies),
            # ACT moves PSUM->f16, DVE forms the m3 chunk. Stage B(j):
            # U-contraction, weight mul, group sum. Emitting A(j+1) before
            # B(j) keeps PE busy during A(j)'s ACT/DVE round trip.
            # Fully software-pipelined loop over 512-col chunks, 5 stages
            # skewed so that within a block every instruction consumes only
            # previous-block results (no intra-block cross-engine chains).
            # no_sync_barrier between blocks pins the schedule to this
            # interleaving; engines still overlap freely across blocks
            # because the fences add no semaphores.
            sel_ps = {}
            d_ps = {}
            dw_sb = {}
            pt_ps = {}

            def st_sel(j):                      # PE
                sab = ps_s.tile([N3 - M3SPLIT, 2 * ft], mybir.dt.float32,
                                name="sab", tag="sab")
                js = slice(j * ft, (j + 1) * ft)
                nc.tensor.matmul(sab[:, 0:ft], lhsT=sb_srepa,
                                 rhs=m2[:, js], start=True, stop=True)
                nc.tensor.matmul(sab[:, ft:2 * ft], lhsT=sb_srepb,
                                 rhs=m2[:, js], start=True, stop=True)
                sel_ps[j] = sab

            def st_copy(j):                     # ACT (same block as st_sel:
                sab = sel_ps.pop(j)                # sab is single-buffered)
                dst = bass.AP(tensor=m2rep.tensor, offset=j * ft,
                              ap=[[2 * F, N3 - M3SPLIT], [F, 2], [1, ft]])
                nc.scalar.activation(dst, sab[:], AF.Copy)

            def st_m3(j):                       # DVE
                js = slice(j * ft, (j + 1) * ft)
                nc.vector.tensor_mul(mon_lo[M3A0:M3A0 + M3SPLIT, js],
                                     m2rep[0:M3SPLIT, js], xrep_a[:, js])
                nc.vector.tensor_mul(mon_hi[:, js],
                                     m2rep[:, F + j * ft:F + (j + 1) * ft],
                                     xrep_b[:, js])

            def st_main(j):                     # PE
                js = slice(j * ft, (j + 1) * ft)
                dlo = ps_d.tile([128, ft], mybir.dt.float32, name="dlo",
                                tag="dlo")
                dhi = ps_d.tile([MOUT - 128, ft], mybir.dt.float32, name="dhi",
                                tag="dhi")
                nc.tensor.matmul(dlo[:], lhsT=sb_uclo[:, 0:128],
                                 rhs=mon_lo[:, js], start=True, stop=False)
                nc.tensor.matmul(dlo[:], lhsT=sb_uchi[:, 0:128],
                                 rhs=mon_hi[:, js], start=False, stop=True)
                nc.tensor.matmul(dhi[:], lhsT=sb_uclo[:, 128:MOUT],
                                 rhs=mon_lo[:, js], start=True, stop=False)
                nc.tensor.matmul(dhi[:], lhsT=sb_uchi[:, 128:MOUT],
                                 rhs=mon_hi[:, js], start=False, stop=True)
                d_ps[j] = (dlo, dhi)

            def st_dw(j):                       # DVE
                js = slice(j * ft, (j + 1) * ft)
                dlo, dhi = d_ps.pop(j)
                dw_lo = work.tile([128, ft], f16, name="dw_lo", tag="dw_lo")
                dw_hi = work.tile([MOUT - 128, ft], f16, name="dw_hi",
                                  tag="dw_hi")
                nc.vector.tensor_mul(dw_lo[:], dlo[:], wrep_lo[:, js])
                nc.vector.tensor_mul(dw_hi[:], dhi[:], wrep_hi[:, js])
                dw_sb[j] = (dw_lo, dw_hi)

            def st_g(j):                        # PE
                dw_lo, dw_hi = dw_sb.pop(j)
                pt = ps_t.tile([MD, ft], mybir.dt.float32, name="pt", tag="pt")
                nc.tensor.matmul(pt[:], lhsT=sb_glo, rhs=dw_lo[:],
                                 start=True, stop=False)
                nc.tensor.matmul(pt[:], lhsT=sb_ghi, rhs=dw_hi[:],
                                 start=False, stop=True)
                pt_ps[j] = pt

            def st_term(j):                     # ACT
                js = slice(j * ft, (j + 1) * ft)
                pt = pt_ps.pop(j)
                nc.scalar.activation(termSB[:, js], pt[:], AF.Copy)

            def st_selcopy(j):
                st_sel(j)
                st_copy(j)

            stages = [st_selcopy, st_m3, st_main, st_dw, st_g, st_term]
            nstg = len(stages)
            for blk in range(nft + nstg - 1):
                for s, fn_s in enumerate(stages):
                    j = blk - s
                    if 0 <= j < nft:
                        fn_s(j)
                tc.no_sync_barrier()

            # ---------------- S8: termT[c, (o,n)] one DMA per o, both lanes;
            # the final per-l SO3 linear runs as soon as its o-block of
            # termT is complete (l=0 needs o=0 only, l=1 o=1..3, l=2 o=4..8)
            def s8(o):
                dma(2 if o % 3 == 2 else o % 2,
                    termT[:, o * nloc:(o + 1) * nloc], termSB[o:o + 1, :])

            def final_l(l):
                c0, c1 = lblk[l]
                w_l = sb_wct[:, l * CD:(l + 1) * CD]
                for s0 in range(c0, c1, FT):
                    s1 = min(s0 + FT, c1)
                    pf = ps_t.tile([CD, ft], mybir.dt.float32, name="pf",
                                   tag="pt")
                    nc.tensor.matmul(pf[:, :s1 - s0], lhsT=w_l,
                                     rhs=termT[:, s0:s1], start=True, stop=True)
                    if l == 0:
                        nc.scalar.activation(outSB[:, s0:s1], pf[:, :s1 - s0],
                                             AF.Identity, bias=sb_b2)
                    else:
                        nc.scalar.activation(outSB[:, s0:s1], pf[:, :s1 - s0],
                                             AF.Copy)

            s8(0)
            final_l(0)
            for o in (1, 2, 3):
                s8(o)
            final_l(1)
            for o in (4, 5, 6, 7, 8):
                s8(o)
            final_l(2)
            nc.sync.dma_start(out=out_d, in_=outSB)

          if repeat > 1:
              with tc.For_i(0, repeat, 1):
                  _emit()
          else:
              _emit()

    return nc


def _get_program(nloc, repeat=1, stage=4):
    key = (nloc, repeat, stage)
    if key not in _PROGRAM:
        nc = _build_program(nloc, repeat, stage)
        nc.compile()
        _PROGRAM[key] = nc
    return _PROGRAM[key]


def make_in_maps(irreps_x, atomic_numbers, w_fc1, b_fc1, U3, W3, U2, W2, U1, W1,
                 w_lin, w_fc2, b_fc2, nloc=NLOC, ncores=NCORES):
    irreps_x = np.asarray(irreps_x, np.float32)
    a_n = np.asarray(atomic_numbers).astype(np.int64)
    U3c, U2c = _sym_compress(np.asarray(U3, np.float64),
                             np.asarray(U2, np.float64))
    Ucomb, G = _build_ucomb_g(U3c, U2c, np.asarray(U1, np.float32))
    w_comb = np.einsum('lde,lec->ldc', np.asarray(w_fc2, np.float32),
                       np.asarray(w_lin, np.float32))
    w1t = np.concatenate([np.asarray(w_fc1, np.float32)[l].T for l in range(3)],
                         axis=1)
    wct = np.concatenate([w_comb[l].T for l in range(3)], axis=1)
    w3g = np.asarray(W3, np.float32)[a_n]
    w2g = np.asarray(W2, np.float32)[a_n]
    w1g = np.asarray(W1, np.float32)[a_n]
    F = nloc * CD
    w9 = MD * nloc
    pkw_in = PK_BASE + w9

    def put(buf, nm, arr, r0=0):
        o = PK_OFF[nm]
        arr = np.asarray(arr, np.float32).astype(np.float16)
        buf[r0:r0 + arr.shape[0], o:o + arr.shape[1]] = arr

    uc_p = Ucomb[KPERM]
    # m2rep row t = m2 row srcrow(t); Srep[srcrow(t), t] = 1 (PE gather)
    srep = np.zeros((N2, N3), np.float32)
    t = 0
    for e in range(MD):
        for d in range(MD - e):
            for a in range(MD - d - e):
                srep[DOFF[d] + a, t] = 1.0
                t += 1
    b12 = np.stack([np.asarray(b_fc1, np.float32),
                    np.asarray(b_fc2, np.float32)], axis=1).astype(np.float32)
    in_maps = []
    for core in range(ncores):
        s = slice(core * nloc, (core + 1) * nloc)
        parts = []
        for l in range(3):
            seg = irreps_x[s, l * l:(l + 1) * (l + 1), :]   # [nloc, w, 64]
            parts.append(seg.transpose(2, 1, 0).reshape(CD, -1))
        xt = np.concatenate(parts, axis=1)                  # [64, 9*nloc]
        pk = np.zeros((128, pkw_in), np.float16)
        put(pk, "uclo", uc_p)              # 118 rows; 119.. stay zero
        put(pk, "uchi", Ucomb[ZR:KMON])
        put(pk, "glo", G[0:128])
        put(pk, "ghi", G[128:MOUT])
        put(pk, "w1t", w1t)
        put(pk, "wct", wct)
        put(pk, "srepa", srep[:, 0:M3SPLIT], r0=M2R)
        put(pk, "srepb", srep[:, M3SPLIT:N3], r0=M2R)
        pk[:CD, PK_BASE:PK_BASE + w9] = xt.astype(np.float16)
        wg = np.concatenate([
            w3g[s].transpose(1, 2, 0).reshape(P3D, F),
            w2g[s].transpose(1, 2, 0).reshape(P2D, F),
            w1g[s].transpose(1, 2, 0).reshape(P1D, F),
        ], axis=0)                                          # [22, F] f=c*nloc+n
        in_maps.append({
            "pk": pk,
            "b12": b12,
            "wg": wg.astype(np.float16),
            "z": np.zeros((1, F), np.float16),
        })
    return in_maps


def unpack_out(o, nloc=NLOC):
    # o: [64, 9*nloc] cols (o, n) o-major -> [nloc, 9, 64]
    return np.ascontiguousarray(
        o.reshape(CD, MD, nloc).transpose(2, 1, 0)).astype(np.float32)


# ---------------------------------------------------------------- entry
def kernel(**inputs):
    from concourse import bass_utils
    in_maps = make_in_maps(**inputs)
    nc = _get_program(NLOC)
    res = bass_utils.run_bass_kernel_spmd(nc, in_maps,
                                          core_ids=list(range(NCORES)))
    outs = [unpack_out(res.results[c]["out"]) for c in range(NCORES)]
    return np.concatenate(outs, axis=0).astype(np.float32)



# revision 63
# speedup vs baseline: 3.0558x; 1.0516x over previous
"""Trainium2 Bass kernel for the MACE 3-body symmetric-contraction block.

Math (equal to the reference einsum chain, validated in factor_check.py):
  1. fc1: per-l SO3 linear on irreps_x -> x[n,m,c]  (bias on l=0)
  2. Per (node n, channel c) the 9-vector x enters a symmetric polynomial:
        out[o] = sum_p w1[n,p,c] sum_i   U1[o,i,p] x_i
               + sum_p w2[n,p,c] sum_ai  U2[o,a,i,p] x_a x_i
               + sum_p w3[n,p,c] sum_abi U3[o,a,b,i,p] x_a x_b x_i
     The x-products are symmetric, so U2/U3 are pre-symmetrized and
     compressed over sorted tuples.  Monomial vector per (n,c):
        mon = [x (9) | m2 = sym x.x (45) | m3 = sym x.x.x (165)]  (219)
     One matmul D[(o,p), f] = Ucomb^T mon  (198x219) does all the
     U-contractions, batched over f = (channel, node).
  3. D * (per-node gathered element weights), p-summed with a 0/1 matmul
     -> term[o, f].
  4. w_lin and w_fc2 fused host-side into one per-l channel mix; final
     SO3 linear + bias.

Monomial/pair orderings are DIAGONAL-major so every replication needed on
device is a partition-aligned prefix/suffix copy (hardware DMA cannot
broadcast SBUF partitions):
  pairs   q=(a, a+d)        ordered by (d, a)
  triples t=(a, a+d, a+d+e) ordered by (e, d, a)
  m2[q]  = x_a * x_{a+d}:      Xpre run x[0:9-d],  Xsuf run x[d:9]
  m3[t]  = m2[(a,a+d)] * x_i:  m2rep run m2[DOFF[d]:+9-d-e], Xrep x[d+e:9]
The e-block of Xrep equals the row-suffix Xsuf[DOFF[e]:45], so Xrep needs
only 9 DMAs.  The 37 m2rep prefix runs are NOT DMA'd: a 0/1 gather matrix
(srep) replays them as a PE selection matmul per 512-column chunk, with
one ACT op moving both PSUM halves to SBUF f16 and DVE forming the m3
chunk — all fused into a 6-stage software pipeline with the main
U-contraction / weight / group-sum chain (no_sync_barrier pins the
schedule; stages are skewed so each block only consumes previous-block
results).  Remaining DMAs are split across the HWDGE rings and SWDGE.

Sharding: pure node-parallel, 128 nodes/core on 8 cores (SPMD, no
collectives).  Host does marshaling only (gathers/transposes/casts).
"""

import numpy as np

# ---------------------------------------------------------------- constants
NTOT, MD, CD = 1024, 9, 64
P3D, P2D, P1D = 16, 4, 2
NCORES = 8
NLOC = NTOT // NCORES              # 128 nodes per core
FT = 512                           # f-tile width for the main matmul

PAIRS = [(a, a + d) for d in range(MD) for a in range(MD - d)]          # 45
TRIPS = [(a, a + d, a + d + e) for e in range(MD) for d in range(MD - e)
         for a in range(MD - d - e)]                                    # 165
N2, N3 = len(PAIRS), len(TRIPS)
DOFF = [0]
for d in range(MD):
    DOFF.append(DOFF[-1] + (MD - d))   # diag-block offsets in PAIRS
KMON = MD + N2 + N3                # 219 monomial rows
MOUT = MD * (P3D + P2D + P1D)      # 198 output rows of the main matmul
MSPLIT = 128                       # k/out split point
M3SPLIT = 64                       # m3 rows below this live in mon_lo

# mon_lo row layout: [m3a (64) | m2 (45) | x (9) | zeros (10)];
# mon_hi = m3[64:165] (101 rows). Compute-engine APs need 32-aligned base
# partitions and matmul needs lhsT.base == rhs.base, so m2 sits at 64 (as
# do srep in pk and xpre/xsuf in their tiles) and m3a at 0. The 10 zero
# rows pair with zero Ucomb rows (filled once from a zero DRAM row).
KPERM = list(range(54, 118)) + list(range(9, 54)) + list(range(0, 9))
M3A0 = 0                           # m3a rows start in mon_lo
M2R = 64                           # m2 rows start in mon_lo
XR = 109                           # x rows start in mon_lo
ZR = 118                           # zero rows start in mon_lo

# packed fp16 constant layout (columns); xt/ysb/termT widths scale w/ nloc
W9 = MD * NLOC
_PK_ITEMS = (("w1t", 3 * CD), ("uclo", MOUT), ("uchi", MOUT), ("glo", MD),
             ("ghi", MD), ("wct", 3 * CD),
             ("srepa", N3 - M3SPLIT), ("srepb", N3 - M3SPLIT))
PK_OFF = {}
_c = 0
for _nm, _w in _PK_ITEMS:
    PK_OFF[_nm] = _c
    _c += _w
PK_BASE = _c                       # xt starts here

_PROGRAM = {}                      # (nloc, repeat) -> compiled Bass program


# ---------------------------------------------------------------- host prep
def _sym_compress(U3, U2):
    tidx = {t: k for k, t in enumerate(TRIPS)}
    qidx = {q: k for k, q in enumerate(PAIRS)}
    U3c = np.zeros((MD, N3, P3D), np.float64)
    for a in range(MD):
        for b in range(MD):
            for i in range(MD):
                U3c[:, tidx[tuple(sorted((a, b, i)))], :] += U3[:, a, b, i, :]
    U2c = np.zeros((MD, N2, P2D), np.float64)
    for a in range(MD):
        for i in range(MD):
            U2c[:, qidx[tuple(sorted((a, i)))], :] += U2[:, a, i, :]
    return U3c.astype(np.float32), U2c.astype(np.float32)


def _build_ucomb_g(U3c, U2c, U1):
    # Ucomb[k, mo]; mon rows: 0..8 x | 9..53 m2 | 54..218 m3
    # out rows mo:  o*16+p (corr3) | 144+o*4+p (corr2) | 180+o*2+p (corr1)
    U = np.zeros((KMON, MOUT), np.float32)
    G = np.zeros((MOUT, MD), np.float32)
    for o in range(MD):
        U[54:54 + N3, o * P3D:(o + 1) * P3D] = U3c[o]
        U[9:9 + N2, 144 + o * P2D:144 + (o + 1) * P2D] = U2c[o]
        U[0:MD, 180 + o * P1D:180 + (o + 1) * P1D] = U1[o]
        G[o * P3D:(o + 1) * P3D, o] = 1.0
        G[144 + o * P2D:144 + (o + 1) * P2D, o] = 1.0
        G[180 + o * P1D:180 + (o + 1) * P1D, o] = 1.0
    return U, G


# ---------------------------------------------------------------- device
def _build_program(nloc, repeat=1, stage=4):
    import concourse.bacc as bacc
    import concourse.bass as bass
    from concourse import mybir
    from concourse.tile import TileContext

    f16 = mybir.dt.float16
    f32 = mybir.dt.float32
    AF = mybir.ActivationFunctionType
    F = nloc * CD
    nft = max(1, F // FT)
    ft = min(FT, F)
    w9 = MD * nloc
    lblk = [(0, nloc), (nloc, 4 * nloc), (4 * nloc, 9 * nloc)]
    pkw_in = PK_BASE + w9              # loaded region of the pack

    nc = bacc.Bacc("TRN2", debug=False, enable_asserts=False,
                   num_devices=NCORES, num_swdge_queues=4)

    pk_d = nc.dram_tensor("pk", [128, pkw_in], f16, kind="ExternalInput").ap()
    b12_d = nc.dram_tensor("b12", [CD, 2], f32, kind="ExternalInput").ap()
    wg_d = nc.dram_tensor("wg", [P3D + P2D + P1D, F], f16,
                          kind="ExternalInput").ap()
    z_d = nc.dram_tensor("z", [1, F], f16, kind="ExternalInput").ap()
    out_d = nc.dram_tensor("out", [CD, w9], f32, kind="ExternalOutput").ap()

    with TileContext(nc) as tc:
        with (
            tc.tile_pool(name="const", bufs=2) as const,
            tc.tile_pool(name="big", bufs=1) as big,
            tc.tile_pool(name="work", bufs=2) as work,
            tc.tile_pool(name="ps_s", bufs=1, space="PSUM") as ps_s,
            tc.tile_pool(name="ps_d", bufs=2, space="PSUM") as ps_d,
            tc.tile_pool(name="ps_t", bufs=2, space="PSUM") as ps_t,
        ):
          # explicit queue spreader: 0 = SP ring, 1 = ACT ring, 2 = SWDGE
          def dma(q, out, in_):
              if q == 0:
                  nc.sync.dma_start(out=out, in_=in_)
              elif q == 1:
                  nc.scalar.dma_start(out=out, in_=in_)
              else:
                  nc.gpsimd.dma_start(out=out, in_=in_)

          def _emit():
            pk = const.tile([128, pkw_in], f16)
            sb_uclo = pk[0:128, PK_OFF["uclo"]:PK_OFF["uclo"] + MOUT]
            sb_uchi = pk[0:N3 - M3SPLIT, PK_OFF["uchi"]:PK_OFF["uchi"] + MOUT]
            sb_glo = pk[0:128, PK_OFF["glo"]:PK_OFF["glo"] + MD]
            sb_ghi = pk[0:MOUT - 128, PK_OFF["ghi"]:PK_OFF["ghi"] + MD]
            sb_w1t = pk[0:CD, PK_OFF["w1t"]:PK_OFF["w1t"] + 3 * CD]
            sb_wct = pk[0:CD, PK_OFF["wct"]:PK_OFF["wct"] + 3 * CD]
            nhr = N3 - M3SPLIT
            sb_srepa = pk[M2R:M2R + N2,
                          PK_OFF["srepa"]:PK_OFF["srepa"] + nhr]
            sb_srepb = pk[M2R:M2R + N2,
                          PK_OFF["srepb"]:PK_OFF["srepb"] + nhr]
            sb_xt = pk[0:CD, PK_BASE:PK_BASE + w9]
            ysb = const.tile([CD, w9], f16, bufs=1)
            termT = const.tile([CD, w9], f16, bufs=1)
            pk32 = const.tile([CD, 2 + w9], f32)
            sb_b1 = pk32[:, 0:1]
            sb_b2 = pk32[:, 1:2]
            outSB = pk32[:, 2:2 + w9]

            mon_lo = big.tile([128, F], f16)
            mon_hi = big.tile([N3 - M3SPLIT, F], f16)
            xpre = big.tile([M2R + N2, F], f16)
            xsuf = big.tile([M2R + N2, F], f16)
            # m2rep halves side by side in one tile: cols [0:F) hold the
            # 64-row a-half, [F:2F) the 101-row b-half, so one ACT op can
            # copy both sel results per chunk.
            m2rep = big.tile([N3 - M3SPLIT, 2 * F], f16)
            xrep_a = big.tile([M3SPLIT, F], f16)
            xrep_b = big.tile([N3 - M3SPLIT, F], f16)
            wrep_lo = big.tile([128, F], f16)
            wrep_hi = big.tile([MOUT - 128, F], f16)
            termSB = big.tile([MD, F], f16)

            # t=0 loads. wrep is NOT loaded here: its 3.2MB would hog the
            # DMA-data slot and push back S3/xsuf completions; it is only
            # needed by the first dw mul, so it loads during the copy phase.
            # pk in three pieces: w1t + xt first so fc1 starts ~1us earlier
            nc.sync.dma_start(out=pk[:, 0:3 * CD], in_=pk_d[:, 0:3 * CD])
            nc.sync.dma_start(out=pk[:, PK_BASE:pkw_in],
                              in_=pk_d[:, PK_BASE:pkw_in])
            nc.sync.dma_start(out=pk[:, 3 * CD:PK_BASE],
                              in_=pk_d[:, 3 * CD:PK_BASE])
            nc.scalar.dma_start(out=pk32[:, 0:2], in_=b12_d)

            # ---------------- S2: fc1 per l (cols m-major within l block)
            for l, (c0, c1) in enumerate(lblk):
                w_l = sb_w1t[:, l * CD:(l + 1) * CD]
                for s0 in range(c0, c1, FT):
                    s1 = min(s0 + FT, c1)
                    py = ps_t.tile([CD, ft], mybir.dt.float32, name="py",
                                   tag="pt")
                    nc.tensor.matmul(py[:, :s1 - s0], lhsT=w_l,
                                     rhs=sb_xt[:, s0:s1], start=True, stop=True)
                    if l == 0:
                        nc.scalar.activation(ysb[:, s0:s1], py[:, :s1 - s0],
                                             AF.Identity, bias=sb_b1)
                    else:
                        nc.scalar.activation(ysb[:, s0:s1], py[:, :s1 - s0],
                                             AF.Copy)

            # S3: x rows — each m is a contiguous [64, nloc] slab of ysb.
            # SP ring + SWDGE only: the ACT sequencer must stay free to run
            # the ysb activations these DMAs wait on (head-of-line blocking).
            for m in range(MD):
                src = ysb[:, m * nloc:(m + 1) * nloc]
                dma(2 if m % 2 else 0, mon_lo[XR + m:XR + m + 1, :], src)

            if stage <= 1:
                nc.sync.dma_start(out=out_d, in_=outSB)
                return
            # zero rows of mon_lo (pair with zero Ucomb rows)
            zsrc = bass.AP(tensor=z_d.tensor, offset=0,
                           ap=[[0, 128 - ZR], [1, F]])
            nc.scalar.dma_start(out=mon_lo[ZR:128, :], in_=zsrc)

            # S4a: xsuf/xpre (based at partition 64, like m2); the HWDGE
            # lane is ~0.63us/DMA vs SWDGE ~1us, so give it the larger share
            _q = [0, 1, 2, 0, 1]               # 4:1 HWDGE:SWDGE
            for d in range(MD):
                q0, cnt = M2R + DOFF[d], MD - d
                dma(_q[(2 * d) % 5], xsuf[q0:q0 + cnt, :],
                    mon_lo[XR + d:XR + MD, :])
                dma(_q[(2 * d + 1) % 5], xpre[q0:q0 + cnt, :],
                    mon_lo[XR:XR + cnt, :])

            # xrep: e block == xsuf[DOFF[e]:45] — SP ring + SWDGE
            xrep_runs = []
            t0 = 0
            for e in range(MD):
                ln = N2 - DOFF[e]
                xrep_runs.append((t0, DOFF[e], ln))
                t0 += ln
            qi = 0
            for t0, s0, ln in xrep_runs:
                segs = []
                if t0 < M3SPLIT:
                    n_lo = min(ln, M3SPLIT - t0)
                    segs.append((xrep_a, t0, s0, n_lo))
                    if n_lo < ln:
                        segs.append((xrep_b, 0, s0 + n_lo, ln - n_lo))
                else:
                    segs.append((xrep_b, t0 - M3SPLIT, s0, ln))
                for dst, dt0, ds0, dln in segs:
                    dma([0, 2][qi % 2], dst[dt0:dt0 + dln, :],
                        xsuf[M2R + ds0:M2R + ds0 + dln, :])
                    qi += 1

            # wrep — 4 DMAs with step-0 DRAM source; first needed by dw(j=0)
            src8 = bass.AP(tensor=wg_d.tensor, offset=0,
                           ap=[[0, 8], [F, 16], [1, F]])
            nc.sync.dma_start(out=wrep_lo[:], in_=src8)
            nc.scalar.dma_start(out=wrep_hi[0:16, :], in_=wg_d[0:16, :])
            src2 = bass.AP(tensor=wg_d.tensor, offset=16 * F,
                           ap=[[0, MD], [F, 4], [1, F]])
            nc.sync.dma_start(out=wrep_hi[16:52, :], in_=src2)
            src1 = bass.AP(tensor=wg_d.tensor, offset=20 * F,
                           ap=[[0, MD], [F, 2], [1, F]])
            nc.scalar.dma_start(out=wrep_hi[52:70, :], in_=src1)

            # m2 written straight into its mon_lo slot
            # column quarters so the first sel chunks start earlier
            m2 = mon_lo[M2R:M2R + N2, :]
            fq = F // 4
            for qq in range(4):
                qs = slice(qq * fq, (qq + 1) * fq)
                nc.vector.tensor_mul(m2[:, qs], xpre[M2R:M2R + N2, qs],
                                     xsuf[M2R:M2R + N2, qs])

            if stage <= 2:
                nc.sync.dma_start(out=out_d, in_=outSB)
                return
            # ---------------- fused, software-pipelined main loop over
            # 512-col chunks. Stage A(j): m2rep via PE selection matmul
            # (srep is a 0/1 gather matrix replacing 37 SBUF->SBUF copies),
            # ACT moves PSUM->f16, DVE forms the m3 chunk. Stage B(j):
            # U-contraction, weight mul, group sum. Emitting A(j+1) before
            # B(j) keeps PE busy during A(j)'s ACT/DVE round trip.
            # Fully software-pipelined loop over 512-col chunks, 5 stages
            # skewed so that within a block every instruction consumes only
            # previous-block results (no intra-block cross-engine chains).
            # no_sync_barrier between blocks pins the schedule to this
            # interleaving; engines still overlap freely across blocks
            # because the fences add no semaphores.
            sel_ps = {}
            d_ps = {}
            dw_sb = {}
            pt_ps = {}

            def st_sel(j):                      # PE
                sab = ps_s.tile([N3 - M3SPLIT, 2 * ft], mybir.dt.float32,
                                name="sab", tag="sab")
                js = slice(j * ft, (j + 1) * ft)
                nc.tensor.matmul(sab[:, 0:ft], lhsT=sb_srepa,
                                 rhs=m2[:, js], start=True, stop=True)
                nc.tensor.matmul(sab[:, ft:2 * ft], lhsT=sb_srepb,
                                 rhs=m2[:, js], start=True, stop=True)
                sel_ps[j] = sab

            def st_copy(j):                     # ACT (same block as st_sel:
                sab = sel_ps.pop(j)                # sab is single-buffered)
                dst = bass.AP(tensor=m2rep.tensor, offset=j * ft,
                              ap=[[2 * F, N3 - M3SPLIT], [F, 2], [1, ft]])
                nc.scalar.activation(dst, sab[:], AF.Copy)

            def st_m3(j):                       # DVE
                js = slice(j * ft, (j + 1) * ft)
                nc.vector.tensor_mul(mon_lo[M3A0:M3A0 + M3SPLIT, js],
                                     m2rep[0:M3SPLIT, js], xrep_a[:, js])
                nc.vector.tensor_mul(mon_hi[:, js],
                                     m2rep[:, F + j * ft:F + (j + 1) * ft],
                                     xrep_b[:, js])

            def st_main(j):                     # PE
                js = slice(j * ft, (j + 1) * ft)
                dlo = ps_d.tile([128, ft], mybir.dt.float32, name="dlo",
                                tag="dlo")
                dhi = ps_d.tile([MOUT - 128, ft], mybir.dt.float32, name="dhi",
                                tag="dhi")
                nc.tensor.matmul(dlo[:], lhsT=sb_uclo[:, 0:128],
                                 rhs=mon_lo[:, js], start=True, stop=False)
                nc.tensor.matmul(dlo[:], lhsT=sb_uchi[:, 0:128],
                                 rhs=mon_hi[:, js], start=False, stop=True)
                nc.tensor.matmul(dhi[:], lhsT=sb_uclo[:, 128:MOUT],
                                 rhs=mon_lo[:, js], start=True, stop=False)
                nc.tensor.matmul(dhi[:], lhsT=sb_uchi[:, 128:MOUT],
                                 rhs=mon_hi[:, js], start=False, stop=True)
                d_ps[j] = (dlo, dhi)

            def st_dw(j):                       # DVE
                js = slice(j * ft, (j + 1) * ft)
                dlo, dhi = d_ps.pop(j)
                dw_lo = work.tile([128, ft], f16, name="dw_lo", tag="dw_lo")
                dw_hi = work.tile([MOUT - 128, ft], f16, name="dw_hi",
                                  tag="dw_hi")
                nc.vector.tensor_mul(dw_lo[:], dlo[:], wrep_lo[:, js])
                nc.vector.tensor_mul(dw_hi[:], dhi[:], wrep_hi[:, js])
                dw_sb[j] = (dw_lo, dw_hi)

            def st_g(j):                        # PE
                dw_lo, dw_hi = dw_sb.pop(j)
                pt = ps_t.tile([MD, ft], mybir.dt.float32, name="pt", tag="pt")
                nc.tensor.matmul(pt[:], lhsT=sb_glo, rhs=dw_lo[:],
                                 start=True, stop=False)
                nc.tensor.matmul(pt[:], lhsT=sb_ghi, rhs=dw_hi[:],
                                 start=False, stop=True)
                pt_ps[j] = pt

            def st_term(j):                     # ACT
                js = slice(j * ft, (j + 1) * ft)
                pt = pt_ps.pop(j)
                nc.scalar.activation(termSB[:, js], pt[:], AF.Copy)

            def st_selcopy(j):
                st_sel(j)
                st_copy(j)

            stages = [st_selcopy, st_m3, st_main, st_dw, st_g, st_term]
            nstg = len(stages)
            for blk in range(nft + nstg - 1):
                for s, fn_s in enumerate(stages):
                    j = blk - s
                    if 0 <= j < nft:
                        fn_s(j)
                tc.no_sync_barrier()

            # ---------------- S8: termT[c, (o,n)] one DMA per o, both lanes;
            # the final per-l SO3 linear runs as soon as its o-block of
            # termT is complete (l=0 needs o=0 only, l=1 o=1..3, l=2 o=4..8)
            def s8(o):
                dma(2 if o % 3 == 2 else o % 2,
                    termT[:, o * nloc:(o + 1) * nloc], termSB[o:o + 1, :])

            def final_l(l):
                c0, c1 = lblk[l]
                w_l = sb_wct[:, l * CD:(l + 1) * CD]
                for s0 in range(c0, c1, FT):
                    s1 = min(s0 + FT, c1)
                    pf = ps_t.tile([CD, ft], mybir.dt.float32, name="pf",
                                   tag="pt")
                    nc.tensor.matmul(pf[:, :s1 - s0], lhsT=w_l,
                                     rhs=termT[:, s0:s1], start=True, stop=True)
                    if l == 0:
                        nc.scalar.activation(outSB[:, s0:s1], pf[:, :s1 - s0],
                                             AF.Identity, bias=sb_b2)
                    else:
                        nc.scalar.activation(outSB[:, s0:s1], pf[:, :s1 - s0],
                                             AF.Copy)

            s8(0)
            final_l(0)
            for o in (1, 2, 3):
                s8(o)
            final_l(1)
            for o in (4, 5, 6, 7, 8):
                s8(o)
            final_l(2)
            nc.sync.dma_start(out=out_d, in_=outSB)

          if repeat > 1:
              with tc.For_i(0, repeat, 1):
                  _emit()
          else:
              _emit()

    return nc


def _get_program(nloc, repeat=1, stage=4):
    key = (nloc, repeat, stage)
    if key not in _PROGRAM:
        nc = _build_program(nloc, repeat, stage)
        nc.compile()
        _PROGRAM[key] = nc
    return _PROGRAM[key]


def make_in_maps(irreps_x, atomic_numbers, w_fc1, b_fc1, U3, W3, U2, W2, U1, W1,
                 w_lin, w_fc2, b_fc2, nloc=NLOC, ncores=NCORES):
    irreps_x = np.asarray(irreps_x, np.float32)
    a_n = np.asarray(atomic_numbers).astype(np.int64)
    U3c, U2c = _sym_compress(np.asarray(U3, np.float64),
                             np.asarray(U2, np.float64))
    Ucomb, G = _build_ucomb_g(U3c, U2c, np.asarray(U1, np.float32))
    w_comb = np.einsum('lde,lec->ldc', np.asarray(w_fc2, np.float32),
                       np.asarray(w_lin, np.float32))
    w1t = np.concatenate([np.asarray(w_fc1, np.float32)[l].T for l in range(3)],
                         axis=1)
    wct = np.concatenate([w_comb[l].T for l in range(3)], axis=1)
    w3g = np.asarray(W3, np.float32)[a_n]
    w2g = np.asarray(W2, np.float32)[a_n]
    w1g = np.asarray(W1, np.float32)[a_n]
    F = nloc * CD
    w9 = MD * nloc
    pkw_in = PK_BASE + w9

    def put(buf, nm, arr, r0=0):
        o = PK_OFF[nm]
        arr = np.asarray(arr, np.float32).astype(np.float16)
        buf[r0:r0 + arr.shape[0], o:o + arr.shape[1]] = arr

    uc_p = Ucomb[KPERM]
    # m2rep row t = m2 row srcrow(t); Srep[srcrow(t), t] = 1 (PE gather)
    srep = np.zeros((N2, N3), np.float32)
    t = 0
    for e in range(MD):
        for d in range(MD - e):
            for a in range(MD - d - e):
                srep[DOFF[d] + a, t] = 1.0
                t += 1
    b12 = np.stack([np.asarray(b_fc1, np.float32),
                    np.asarray(b_fc2, np.float32)], axis=1).astype(np.float32)
    in_maps = []
    for core in range(ncores):
        s = slice(core * nloc, (core + 1) * nloc)
        parts = []
        for l in range(3):
            seg = irreps_x[s, l * l:(l + 1) * (l + 1), :]   # [nloc, w, 64]
            parts.append(seg.transpose(2, 1, 0).reshape(CD, -1))
        xt = np.concatenate(parts, axis=1)                  # [64, 9*nloc]
        pk = np.zeros((128, pkw_in), np.float16)
        put(pk, "uclo", uc_p)              # 118 rows; 119.. stay zero
        put(pk, "uchi", Ucomb[ZR:KMON])
        put(pk, "glo", G[0:128])
        put(pk, "ghi", G[128:MOUT])
        put(pk, "w1t", w1t)
        put(pk, "wct", wct)
        put(pk, "srepa", srep[:, 0:M3SPLIT], r0=M2R)
        put(pk, "srepb", srep[:, M3SPLIT:N3], r0=M2R)
        pk[:CD, PK_BASE:PK_BASE + w9] = xt.astype(np.float16)
        wg = np.concatenate([
            w3g[s].transpose(1, 2, 0).reshape(P3D, F),
            w2g[s].transpose(1, 2, 0).reshape(P2D, F),
            w1g[s].transpose(1, 2, 0).reshape(P1D, F),
        ], axis=0)                                          # [22, F] f=c*nloc+n
        in_maps.append({
            "pk": pk,
            "b12": b12,
            "wg": wg.astype(np.float16),
            "z": np.zeros((1, F), np.float16),
        })
    return in_maps


def unpack_out(o, nloc=NLOC):
    # o: [64, 9*nloc] cols (o, n) o-major -> [nloc, 9, 64]
    return np.ascontiguousarray(
        o.reshape(CD, MD, nloc).transpose(2, 1, 0)).astype(np.float32)


# ---------------------------------------------------------------- entry
def kernel(**inputs):
    from concourse import bass_utils
    in_maps = make_in_maps(**inputs)
    nc = _get_program(NLOC)
    res = bass_utils.run_bass_kernel_spmd(nc, in_maps,
                                          core_ids=list(range(NCORES)))
    outs = [unpack_out(res.results[c]["out"]) for c in range(NCORES)]
    return np.concatenate(outs, axis=0).astype(np.float32)

